# revision 3
# baseline (speedup 1.0000x reference)
"""Trainium2 Bass kernel for batched per-point channel-vector gather.

Problem: imgs [B=8, C=128, H=512, W=512] f32, batch_points [B=8, P=2048, 2]
(x, y) int. Output row (b*P + p) = imgs[b, :, y, x]  ->  [B*P, C] f32.

Sharding: data-parallel over the batch axis, one image per NeuronCore
(8 cores); per-core results are concatenated along the point axis.

Device strategy (per core): a point's C-vector is strided (stride H*W*4 =
1 MiB) in the native [C, H, W] layout, and HW indirect DMA only moves
contiguous runs with a deterministic pairing for the one-index-per-
partition form. So: stream the image through SBUF in [128, 8192] chunks,
PE-transpose each 128-column block (positions onto partitions), write the
transposed chunks to a DRAM scratch laid out as rows of C contiguous
floats, then gather the 2048 query rows with 16 indirect DMAs (idx
[128,1], the production embedding-lookup form) straight into output order.
"""

import numpy as np

B, C, H, W, P = 8, 128, 512, 512, 2048
HW = H * W
T = 8192
NCHUNKS = HW // T
NBLK = T // 128
J = P // 128

_CACHED_NC = None


def _build_nc():
    import concourse.bass as bass
    import concourse.bacc as bacc
    import concourse.mybir as mybir
    from concourse import tile

    nc = bacc.Bacc("TRN2", target_bir_lowering=False, debug=False)
    img = nc.dram_tensor("img", [C, HW], mybir.dt.float32, kind="ExternalInput")
    gidx = nc.dram_tensor("gidx", [128, J], mybir.dt.int32, kind="ExternalInput")
    ident = nc.dram_tensor("ident", [128, 128], mybir.dt.float32, kind="ExternalInput")
    out = nc.dram_tensor("out", [P, C], mybir.dt.float32, kind="ExternalOutput")
    scr = nc.dram_tensor("scr", [HW, C], mybir.dt.float32, kind="Internal")

    with tile.TileContext(nc) as tc:
        with (
            tc.tile_pool(name="io", bufs=1) as iop,
            tc.tile_pool(name="ck", bufs=2) as ckp,
            tc.tile_pool(name="st", bufs=2) as stp,
            tc.tile_pool(name="ps", bufs=4, space="PSUM") as psp,
            tc.tile_pool(name="g", bufs=4) as gp,
        ):
            idx_tile = iop.tile([128, J], mybir.dt.int32)
            nc.sync.dma_start(idx_tile[:], gidx.ap())
            id_tile = iop.tile([128, 128], mybir.dt.float32)
            nc.sync.dma_start(id_tile[:], ident.ap())
            for ci in range(NCHUNKS):
                chunk = ckp.tile([128, T], mybir.dt.float32, tag="chunk")
                nc.sync.dma_start(chunk[:], img.ap()[:, ci * T : (ci + 1) * T])
                staging = stp.tile([128, T], mybir.dt.float32, tag="stg")
                for q in range(T // 512):
                    ps = psp.tile([128, 512], mybir.dt.float32, tag="ps")
                    for j in range(4):
                        c0 = q * 512 + j * 128
                        nc.tensor.transpose(
                            out=ps[:, j * 128 : (j + 1) * 128],
                            in_=chunk[:, c0 : c0 + 128],
                            identity=id_tile[:],
                        )
                    cp = nc.vector.tensor_copy if q % 2 == 0 else nc.scalar.copy
                    cp(staging[:, q * 512 : (q + 1) * 512], ps[:])
                # scratch row r = ci*T + p*NBLK + blk holds the C-vector of
                # position ci*T + blk*128 + p (8 KiB contiguous per partition)
                dst = bass.AP(
                    scr, ci * T * C, [[NBLK * C, 128], [C, NBLK], [1, C]]
                )
                nc.sync.dma_start(dst, staging[:].rearrange("p (b c) -> p b c", c=C))
            for k in range(J):
                g = gp.tile([128, C], mybir.dt.float32, tag="g")
                nc.gpsimd.indirect_dma_start(
                    out=g[:],
                    out_offset=None,
                    in_=scr.ap(),
                    in_offset=bass.IndirectOffsetOnAxis(
                        ap=idx_tile[:, k : k + 1], axis=0
                    ),
                )
                nc.sync.dma_start(out.ap()[k * 128 : (k + 1) * 128, :], g[:])
    nc.compile()
    return nc


def _get_nc():
    global _CACHED_NC
    if _CACHED_NC is None:
        _CACHED_NC = _build_nc()
    return _CACHED_NC


def _make_in_maps(imgs, batch_points):
    imgs = np.asarray(imgs, dtype=np.float32)
    pts = np.asarray(batch_points).astype(np.int64)
    ident = np.eye(128, dtype=np.float32)
    in_maps = []
    for b in range(B):
        flat = pts[b, :, 1] * W + pts[b, :, 0]  # y*W + x
        ci, rem = np.divmod(flat, T)
        blk, pos = np.divmod(rem, 128)
        rows = (ci * T + pos * NBLK + blk).astype(np.int32)
        gidx = np.ascontiguousarray(rows.reshape(J, 128).T)  # [128, J]
        in_maps.append(
            {
                "img": np.ascontiguousarray(imgs[b].reshape(C, HW)),
                "gidx": gidx,
                "ident": ident,
            }
        )
    return in_maps


def run(imgs, batch_points, trace=False, **kwargs):
    from concourse.bass_utils import run_bass_kernel_spmd

    nc = _get_nc()
    in_maps = _make_in_maps(imgs, batch_points)
    res = run_bass_kernel_spmd(
        nc, in_maps, core_ids=list(range(B)), trace=trace, **kwargs
    )
    out = np.concatenate([r["out"] for r in res.results], axis=0)
    return out, res


def kernel(imgs, batch_points):
    out, _ = run(imgs, batch_points)
    return out, out.shape[0]
